# revision 1
# baseline (speedup 1.0000x reference)
"""ArcFace logits on 8 Trainium2 NeuronCores (Bass, raw engine streams).

out[n, c] = S * cos(theta_nc + M * [c == labels[n]]),  cos from L2-normalized
embeddings [1024, 512] x weight [100000, 512].

Strategy: model-parallel over the class dim (partial-FC).  Classes are
padded/permuted on the host so that every core gets 12800 columns and its
128 label hits land on the diagonal of the first 128x128 output block.
That makes the compiled graph identical on all 8 cores and fully
label-independent: the margin fix is a cheap diagonal extract/rewrite with
an identity mask.  The host only moves data (transpose / permute / gather),
all FLOPs (normalization, matmul, margin trig) run on device.

Matmuls run in float32r (full-rate fp32, ~1.5e-4 rel err).  1/sqrt uses the
Ln/Exp activation tables (one table set, no reloads).
"""

import math

import numpy as np

import concourse.bass as bass
import concourse.mybir as mybir
from concourse.bass_utils import run_bass_kernel_spmd

AF = mybir.ActivationFunctionType
OP = mybir.AluOpType
F32 = mybir.dt.float32
F32R = mybir.dt.float32r
BF16 = mybir.dt.bfloat16

S = 30.0
MARGIN = 0.5
N, D, C = 1024, 512, 100000

NCORES = 8
CS = 12800            # classes per core (padded: 8 * 12800 = 102400)
CPAD = NCORES * CS
F = 512               # matmul free dim / class chunk width
NCHUNK = CS // F      # 25
KD = D // 128         # 4 contraction sub-tiles
NB = N // 128         # 8 row blocks
NTILES = NCHUNK * NB  # 200 output tiles per core
NPS = 6               # main PSUM bank rotation
NWT = 6               # wT chunk buffers
NOUT = 32             # out_sb rotation (4 chunks)

COSM = float(math.cos(MARGIN))
SINM = float(math.sin(MARGIN))


def build_graph():
    nc = bass.Bass(target_bir_lowering=False)

    eT_ext = nc.declare_dram_parameter("eT", [D, N], F32, isOutput=False)
    wT_ext = nc.declare_dram_parameter("wT", [D, CS], F32, isOutput=False)
    ones_ext = nc.declare_dram_parameter("ones", [128], F32, isOutput=False)
    ident_ext = nc.declare_dram_parameter("ident", [128, 128], F32, isOutput=False)
    out_ext = nc.declare_dram_parameter("out", [N, CS], F32, isOutput=True)

    import contextlib

    ctx = contextlib.ExitStack()
    sb = lambda name, shape, dt=F32: ctx.enter_context(nc.sbuf_tensor(name, shape, dt))
    ps = lambda name: ctx.enter_context(nc.psum_tensor(name, [128, F], F32))
    sem = lambda name: ctx.enter_context(nc.semaphore(name))

    with ctx:
        # --- SBUF ---
        eT_sb = sb("eT_sb", [128, KD, N])            # raw e^T
        esq = sb("esq", [128, KD, N], F32R)          # e^2 (rounded)
        eTn = sb("eTn", [128, KD, N], BF16)          # S * e / ||e||
        rsqe_row = sb("rsqe_row", [1, N], F32R)
        rsqe_bc = sb("rsqe_bc", [128, N])
        wt = [sb(f"wt{b}", [128, KD, F], BF16) for b in range(NWT)]
        wsq = [sb(f"wsq{b}", [128, KD, F], F32R) for b in range(3)]
        lnw_tmp = sb("lnw_tmp", [1, F])
        rsqw_row = [sb(f"rsqw_row{b}", [1, F], F32R) for b in range(2)]
        rsqw_bc = [sb(f"rsqw_bc{b}", [128, F]) for b in range(2)]
        out_sb = [sb(f"out_sb{b}", [128, F]) for b in range(NOUT)]
        ones128 = sb("ones128", [128, 1], F32R)
        ones1 = sb("ones1", [1, 128], F32R)
        ident_sb = sb("ident_sb", [128, 128])
        diag_tmp = sb("diag_tmp", [128, 128])
        vdiag = sb("vdiag", [128, 1])
        sqv = sb("sqv", [128, 1])
        lnu = sb("lnu", [128, 1])
        s3v = sb("s3v", [128, 1])
        t1v = sb("t1v", [128, 1])
        fixp = sb("fixp", [128, 1])
        deltap = sb("deltap", [128, 1])
        lnS_b = sb("lnS_b", [1, 1])
        s2_b = sb("s2_b", [128, 1])

        # --- PSUM: 6 main banks + ssq row bank + broadcast bank = 8 ---
        ps_main = [ps(f"ps_main{b}") for b in range(NPS)]
        ps_ssq = ps("ps_ssq")
        ps_bc = ps("ps_bc")

        # --- semaphores ---
        s_const = sem("s_const")
        s_ms = sem("s_ms")
        s_wt = [sem(f"s_wt{b}") for b in range(NWT)]
        s_do2 = [sem("s_do0"), sem("s_do1")]
        s_outdone = sem("s_outdone")
        s_sq = sem("s_sq")
        s_ssqmm = sem("s_ssqmm")
        s_row = sem("s_row")
        s_bcmm = sem("s_bcmm")
        s_bcev = sem("s_bcev")
        s_mmtile = sem("s_mmtile")
        s_evtile = sem("s_evtile")
        s_vg = sem("s_vg")
        s_sfix = sem("s_sfix")
        s_vfix = sem("s_vfix")
        s_esq = sem("s_esq")
        s_essqmm = sem("s_essqmm")
        s_erow = sem("s_erow")
        s_ebcmm = sem("s_ebcmm")
        s_ebcp = sem("s_ebcp")
        s_etn = sem("s_etn")

        with nc.Block() as block:

            @block.gpsimd
            def _(g):
                g.memset(lnS_b[:], float(np.log(S))).then_inc(s_ms, 1)
                g.memset(s2_b[:], float(S * S)).then_inc(s_ms, 1)
                def wt_dma(c):
                    g.dma_start(
                        out=wt[c % NWT][:],
                        in_=wT_ext[:, c * F:(c + 1) * F].rearrange("(ko p) f -> p ko f", p=128),
                    ).then_inc(s_wt[c % NWT], 16)

                wt_dma(0)  # first: unblocks squares(0) -> ssq(0) quickly
                g.dma_start(out=ones128[:], in_=ones_ext[:].rearrange("(p o) -> p o", o=1)).then_inc(s_const, 16)
                g.dma_start(out=ones1[:], in_=ones_ext[:].rearrange("(o p) -> o p", o=1)).then_inc(s_const, 16)
                g.dma_start(out=ident_sb[:], in_=ident_ext[:]).then_inc(s_const, 16)
                g.dma_start(out=eT_sb[:], in_=eT_ext[:].rearrange("(ko p) n -> p ko n", p=128)).then_inc(s_const, 16)
                wt_dma(1)
                for c in range(2, min(NWT, NCHUNK)):
                    wt_dma(c)
                NPAIR = NCHUNK // 2
                for p in range(NPAIR + 1):
                    last = p == NPAIR
                    for cp in (2 * p + NWT, 2 * p + 1 + NWT):
                        if cp < NCHUNK:
                            # buffer cp%NWT freed once its pair-block's mains done
                            done_tiles = ((cp - NWT) // 2 + 1) * 16
                            g.wait_ge(s_mmtile, min(done_tiles, 200))
                            g.dma_start(
                                out=wt[cp % NWT][:],
                                in_=wT_ext[:, cp * F:(cp + 1) * F].rearrange("(ko p) f -> p ko f", p=128),
                            ).then_inc(s_wt[cp % NWT], 16)
                    if p == 0:
                        g.wait_ge(s_vfix, 1)
                    for nb in range(NB):
                        for j in range(1 if last else 2):
                            c = 2 * p + j
                            et = p * 16 + (nb if last else nb * 2 + j)
                            g.wait_ge(s_evtile, et + 1)
                            g.dma_start(
                                out=out_ext[nb * 128:(nb + 1) * 128, c * F:(c + 1) * F],
                                in_=out_sb[et % NOUT][:],
                            ).then_inc(s_do2[p % 2], 16)
                # make sure all output DMAs have landed before the graph ends
                g.wait_ge(s_do2[0], 16 * (16 * 6 + 8))
                g.wait_ge(s_do2[1], 16 * 16 * 6)

            @block.scalar
            def _(s):
                def do_squares(x):
                    s.wait_ge(s_wt[x % NWT], 16 * (x // NWT + 1))
                    if x >= 3:
                        s.wait_ge(s_ssqmm, x - 2)  # wsq[x%3] freed by ssq(x-3)
                    for k in range(KD):
                        ins = s.activation(wsq[x % 3][:, k, :], wt[x % NWT][:, k, :], AF.Square)
                    ins.then_inc(s_sq, 1)

                # wT squares for chunks 0/1 as early as possible
                do_squares(0)
                do_squares(1)
                # e-prep
                s.wait_ge(s_const, 64)
                s.wait_ge(s_ms, 2)
                for k in range(KD):
                    ins = s.activation(esq[:, k, :], eT_sb[:, k, :], AF.Square)
                ins.then_inc(s_esq, 1)
                s.wait_ge(s_essqmm, 1)
                for h in range(2):
                    src = ps_main[h][0:1, :]
                    s.activation(lnw_tmp[:], src, AF.Ln)
                    s.drain()
                    ins = s.activation(rsqe_row[0:1, h * F:(h + 1) * F], lnw_tmp[:], AF.Exp, scale=-0.5, bias=lnS_b[:])
                    s.drain()
                ins.then_inc(s_erow, 1)
                s.wait_ge(s_ebcmm, 1)
                s.activation(rsqe_bc[:, 0:F], ps_main[2][:], AF.Copy)
                s.activation(rsqe_bc[:, F:N], ps_main[3][:], AF.Copy).then_inc(s_ebcp, 1)
                # chunk loop: bcast-evict first (unblocks VectorE), then
                # squares two chunks ahead, then the rsqw row chain
                for c in range(NCHUNK):
                    s.wait_ge(s_ssqmm, c + 1)
                    if c >= 2:
                        s.wait_ge(s_bcmm, c - 1)  # rsqw_row[c%2] free: bcast(c-2) read it
                    s.activation(lnw_tmp[:], ps_ssq[0:1, :], AF.Ln)
                    s.drain()
                    s.activation(rsqw_row[c % 2][:], lnw_tmp[:], AF.Exp, scale=-0.5)
                    s.drain().then_inc(s_row, 1)
                    if c >= 1:
                        s.wait_ge(s_bcmm, c)
                        if c >= 3:
                            # rsqw_bc[(c-1)%2] free once chunk c-3's pair is evicted
                            s.wait_ge(s_evtile, 16 * ((c - 3) // 2 + 1))
                        s.activation(rsqw_bc[(c - 1) % 2][:], ps_bc[:], AF.Copy).then_inc(s_bcev, 1)
                    if c + 2 <= NCHUNK - 1:
                        do_squares(c + 2)
                    if c == 2:
                        # margin fix math on the gathered diagonal [128,1]
                        s.wait_ge(s_vg, 1)
                        s.activation(sqv[:], vdiag[:], AF.Square)
                        s.drain()
                        s.activation(lnu[:], sqv[:], AF.Ln, scale=-1.0, bias=s2_b[:])
                        s.drain()
                        s.activation(s3v[:], lnu[:], AF.Exp, scale=0.5)
                        s.activation(t1v[:], vdiag[:], AF.Copy, scale=COSM)
                        s.drain().then_inc(s_sfix, 1)
                # tail: evict last broadcast
                s.wait_ge(s_bcmm, NCHUNK)
                s.activation(rsqw_bc[(NCHUNK - 1) % 2][:], ps_bc[:], AF.Copy).then_inc(s_bcev, 1)

            @block.tensor
            def _(t):
                t.wait_ge(s_const, 64)  # ones128/ones1 + consts
                # chunk-0 ssq as soon as its squares land (keeps PE warm early)
                t.wait_ge(s_sq, 1)
                for k in range(KD):
                    ins = t.matmul(ps_ssq[0:1, :], lhsT=ones128[:], rhs=wsq[0][:, k, :],
                             start=(k == 0), stop=(k == KD - 1))
                ins.then_inc(s_ssqmm, 1)
                # e-prep: ssq_e rows then rsqe broadcast
                t.wait_ge(s_esq, 1)
                for h in range(2):
                    for k in range(KD):
                        ins = t.matmul(
                            ps_main[h][0:1, :], lhsT=ones128[:],
                            rhs=esq[:, k, h * F:(h + 1) * F],
                            start=(k == 0), stop=(k == KD - 1),
                        )
                ins.then_inc(s_essqmm, 1)
                t.wait_ge(s_erow, 1)
                for h in range(2):
                    ins = t.matmul(ps_main[2 + h][:], lhsT=ones1[:],
                             rhs=rsqe_row[0:1, h * F:(h + 1) * F], start=True, stop=True)
                ins.then_inc(s_ebcmm, 1)
                # chunk-1 ssq (the pair blocks start at chunk 2)
                t.wait_ge(s_sq, 2)
                t.wait_ge(s_row, 1)
                for k in range(KD):
                    ins = t.matmul(ps_ssq[0:1, :], lhsT=ones128[:], rhs=wsq[1][:, k, :],
                             start=(k == 0), stop=(k == KD - 1))
                ins.then_inc(s_ssqmm, 1)
                # wait for eTn + eprep psum drains before main work
                t.wait_ge(s_etn, 1)
                t.wait_ge(s_ebcp, 1)
                def do_ssq(B):
                    t.wait_ge(s_sq, B + 1)
                    t.wait_ge(s_row, B)  # ps_ssq freed by row-chain(B-1)
                    for k in range(KD):
                        ins = t.matmul(ps_ssq[0:1, :], lhsT=ones128[:], rhs=wsq[B % 3][:, k, :],
                                 start=(k == 0), stop=(k == KD - 1))
                    ins.then_inc(s_ssqmm, 1)

                def do_bcast(A):
                    t.wait_ge(s_row, A + 1)
                    if A >= 1:
                        t.wait_ge(s_bcev, A)  # ps_bc freed by bcevict(A-1)
                    t.matmul(ps_bc[:], lhsT=ones1[:], rhs=rsqw_row[A % 2][:], start=True, stop=True).then_inc(s_bcmm, 1)

                # chunk pairs (2p, 2p+1); eviction/tile index et = p*16 + nb*2 + j
                NPAIR = NCHUNK // 2  # 12; chunk 24 handled alone at the end
                for p in range(NPAIR + 1):
                    last = p == NPAIR
                    c0, c1 = 2 * p, 2 * p + 1
                    if c0 + 2 <= NCHUNK - 1:
                        do_ssq(c0 + 2)
                    do_bcast(c0)
                    if not last:
                        do_bcast(c1)
                    for nb in range(NB):
                        if nb == 4 and not last and c1 + 2 <= NCHUNK - 1:
                            do_ssq(c1 + 2)
                        et = p * 16 + (nb if last else nb * 2)
                        if et >= NPS:
                            t.wait_ge(s_evtile, et - NPS + 2)
                        for k in range(KD):
                            mm = t.matmul(
                                ps_main[et % NPS][:],
                                lhsT=eTn[:, k, nb * 128:(nb + 1) * 128],
                                rhs=wt[c0 % NWT][:, k, :],
                                start=(k == 0), stop=(k == KD - 1),
                                skip_group_check=True,
                            )
                            if k == KD - 1:
                                mm.then_inc(s_mmtile, 1)
                            if not last:
                                mm = t.matmul(
                                    ps_main[(et + 1) % NPS][:],
                                    lhsT=eTn[:, k, nb * 128:(nb + 1) * 128],
                                    rhs=wt[c1 % NWT][:, k, :],
                                    start=(k == 0), stop=(k == KD - 1),
                                    skip_group_check=True,
                                )
                                if k == KD - 1:
                                    mm.then_inc(s_mmtile, 1)

            @block.vector
            def _(v):
                # e-prep: eTn = eT * rsqe_bc  (f32r)
                v.wait_ge(s_const, 64)
                v.wait_ge(s_ebcp, 1)
                for k in range(KD):
                    v.tensor_tensor(out=eTn[:, k, :], in0=eT_sb[:, k, :], in1=rsqe_bc[:], op=OP.mult)
                v.engine_nop().then_inc(s_etn, 1)
                NPAIR = NCHUNK // 2
                for p in range(NPAIR + 1):
                    last = p == NPAIR
                    if p == 1:
                        # apply margin on the diagonal of tile (0,0)
                        v.wait_ge(s_sfix, 1)
                        v.scalar_tensor_tensor(fixp[:], s3v[:], -SINM, t1v[:], OP.mult, OP.add)
                        v.drain()
                        v.tensor_tensor(out=deltap[:], in0=fixp[:], in1=vdiag[:], op=OP.subtract)
                        v.drain()
                        v.scalar_tensor_tensor(out_sb[0][:, 0:128], ident_sb[:], deltap[:],
                                               out_sb[0][:, 0:128], OP.mult, OP.add)
                        v.drain().then_inc(s_vfix, 1)
                    for nb in range(NB):
                        for j in range(1 if last else 2):
                            c = 2 * p + j
                            et = p * 16 + (nb if last else nb * 2 + j)
                            v.wait_ge(s_mmtile, et + 1)
                            if nb == 0:
                                v.wait_ge(s_bcev, c + 1)
                            if et >= NOUT:
                                # buffer reused from pair p-2 (same parity): wait for
                                # ALL same-parity out-DMAs issued so far (sound: the
                                # parity sem's issue prefix ends at pair p-2)
                                npar = (p - 2 - (p % 2)) // 2 + 1
                                v.wait_ge(s_do2[p % 2], 16 * 16 * npar)
                            v.tensor_tensor(out=out_sb[et % NOUT][:], in0=ps_main[et % NPS][:],
                                            in1=rsqw_bc[c % 2][:], op=OP.mult)
                            v.engine_nop().then_inc(s_evtile, 1)
                            if et == 0:
                                # extract the label diagonal of the first tile
                                v.drain()
                                v.tensor_tensor(out=diag_tmp[:], in0=out_sb[0][:, 0:128],
                                                in1=ident_sb[:], op=OP.mult)
                                v.drain()
                                v.tensor_reduce(vdiag[:], diag_tmp[:],
                                                mybir.AxisListType.X, OP.add)
                                v.drain().then_inc(s_vg, 1)

    return nc


_GRAPH = None


def _get_graph():
    global _GRAPH
    if _GRAPH is None:
        _GRAPH = build_graph()
    return _GRAPH


def _host_prepare(embeddings, weight, labels):
    """Row/class permutations putting each core's labels on the (0,0) diagonal."""
    labels = np.asarray(labels).astype(np.int64)
    e = np.asarray(embeddings, dtype=np.float32)
    w = np.asarray(weight, dtype=np.float32)

    # fix instance i (row i, class labels[i]) goes to core i//128, column i%128
    first_seen = {}
    extras = []  # (core, col, row, cls) for duplicate label classes
    primary_col = {}  # cls -> (core, col)
    for i in range(N):
        l = int(labels[i])
        m, p = i // 128, i % 128
        if l not in first_seen:
            first_seen[l] = (m, p)
            primary_col[l] = (m, p)
        else:
            extras.append((m, p, i, l))

    labeled = np.zeros(C, dtype=bool)
    labeled[labels] = True
    unlab = np.nonzero(~labeled)[0]

    # column map per core: -1 = padding column
    colmaps = np.full((NCORES, CS), -1, dtype=np.int64)
    for i in range(N):
        colmaps[i // 128, i % 128] = labels[i]
    fill_slots = NCORES * (CS - 128)
    fill = np.full(fill_slots, -1, dtype=np.int64)
    fill[: unlab.size] = unlab
    fill = fill.reshape(NCORES, CS - 128)
    colmaps[:, 128:] = fill

    # bulk-assign validity: skip pad and non-primary duplicate columns
    valid_bulk = colmaps >= 0
    for (m, p, i, l) in extras:
        valid_bulk[m, p] = False

    wTfull = w.T  # [512, 100000] view
    in_maps = []
    row_perms = []
    ones = np.ones(128, dtype=np.float32)
    ident = np.eye(128, dtype=np.float32)
    for m in range(NCORES):
        cm = colmaps[m]
        wt = np.zeros((D, CS), dtype=np.float32)
        vmask = cm >= 0
        wt[:, vmask] = wTfull[:, cm[vmask]]
        wt[0, ~vmask] = 1.0
        rows = np.concatenate([
            np.arange(m * 128, (m + 1) * 128),
            np.delete(np.arange(N), np.s_[m * 128:(m + 1) * 128]),
        ])
        row_perms.append(rows)
        eT = np.ascontiguousarray(e[rows].T)
        in_maps.append({
            "eT": eT,
            "wT": np.ascontiguousarray(wt),
            "ones": ones,
            "ident": ident,
        })
    return in_maps, row_perms, colmaps, valid_bulk, extras


def _assemble(results, row_perms, colmaps, valid_bulk, extras):
    out = np.empty((N, C), dtype=np.float32)
    slabs = []
    for m in range(NCORES):
        slab = results[m]["out"]
        unperm = np.empty_like(slab)
        unperm[row_perms[m]] = slab
        slabs.append(unperm)
        vb = valid_bulk[m]
        out[:, colmaps[m][vb]] = unperm[:, vb]
    for (m, p, i, l) in extras:
        out[i, l] = slabs[m][i, p]
    return out


def kernel(embeddings, weight, labels, _trace=False):
    nc = _get_graph()
    in_maps, row_perms, colmaps, valid_bulk, extras = _host_prepare(
        embeddings, weight, labels
    )
    res = run_bass_kernel_spmd(nc, in_maps, core_ids=list(range(NCORES)), trace=_trace)
    out = _assemble(res.results, row_perms, colmaps, valid_bulk, extras)
    if _trace:
        return out, res
    return out



# revision 8
# speedup vs baseline: 1.2235x; 1.2235x over previous
"""ArcFace logits on 8 Trainium2 NeuronCores (Bass, raw engine streams).

out[n, c] = S * cos(theta_nc + M * [c == labels[n]]),  cos from L2-normalized
embeddings [1024, 512] x weight [100000, 512].

Model-parallel over the class dim (partial-FC): classes are padded/permuted
on the host so every core gets 12800 columns and its 128 label hits land on
the diagonal of output tile (chunk 0, row-block 0).  The compiled graph is
identical on all 8 cores and label-independent.

v2 layout/schedule (vs the fp32-I/O baseline):
  - weights and embeddings are shipped bf16, host-packed so every DMA
    descriptor is 4KB+ contiguous per partition (halves HBM read traffic)
  - output is fp16 (halves HBM write traffic), quad-batched: 4 chunks per
    out buffer -> 4KB descriptors, 56 out DMAs instead of 200
  - the matmul consumes RAW bf16 e and w; both norm scales are applied in
    a single DVE scalar_tensor_tensor at PSUM eviction:
        out = (psum * rsqe[row]) * rsqw[col]
    so nothing on the critical path before the first matmul except DMAs
  - w column norms: DVE squares wt (bf16), PE reduces with an all-ones
    [128,128] lhsT so the PSUM result is already broadcast across all
    partitions; ACT does exp(-0.5*ln(ssq)) straight out of PSUM.  No
    separate broadcast matmuls, no ACT copies.
  - e row norms: ACT Square with accum_out over a row-major bf16 copy of e
    (no PE involvement), exp(-0.5*ln+lnS) -> rsqe[128,8] per-partition
    scalars.  S is folded into rsqe.
  - the odd chunk (24) runs FIRST so the kernel tail is a full pair with
    overlapped eviction/DMA instead of a serialized single chunk.
"""

import math

import numpy as np
import ml_dtypes

import concourse.bass as bass
import concourse.mybir as mybir
from concourse.bass_utils import run_bass_kernel_spmd

AF = mybir.ActivationFunctionType
OP = mybir.AluOpType
F32 = mybir.dt.float32
F16 = mybir.dt.float16
BF16 = mybir.dt.bfloat16

S = 30.0
MARGIN = 0.5
N, D, C = 1024, 512, 100000

NCORES = 8
CS = 12800            # classes per core (padded: 8 * 12800 = 102400)
F = 512               # matmul free dim / class chunk width
NCHUNK = CS // F      # 25
KD = D // 128         # 4 contraction sub-tiles
NB = N // 128         # 8 row blocks
NPS = 6               # main PSUM bank rotation (+2 ssq banks = 8)
NWT = 6               # wt chunk buffers
NWSQ = 3              # wsq buffers
NBC = 4               # rsqw_bc rotation
NOUTQ = 16            # out quad-buffer rotation
NPAIR = (NCHUNK - 1) // 2  # 12 pairs after the leading single chunk

COSM = float(math.cos(MARGIN))
SINM = float(math.sin(MARGIN))

# chunk processing order: odd chunk 24 first, then pairs (0,1),(2,3),...
SEQ = [24] + list(range(24))


def _mains_done_tiles(o):
    """s_mmtile value once all 8 main tiles of seq-chunk o are complete."""
    if o == 0:
        return 8
    return 8 + 16 * ((o - 1) // 2 + 1)


def _evict_done_tiles(o):
    """s_evtile value once all 8 evictions of seq-chunk o are complete."""
    if o == 0:
        return 8
    return 8 + 16 * ((o - 1) // 2 + 1)


def build_graph():
    nc = bass.Bass(target_bir_lowering=False)

    eT_ext = nc.declare_dram_parameter("eT", [128, KD * N], BF16, isOutput=False)
    erow_ext = nc.declare_dram_parameter("erow", [128, NB * D], BF16, isOutput=False)
    w_ext = nc.declare_dram_parameter("w", [128, NCHUNK * KD * F], BF16, isOutput=False)
    ident_ext = nc.declare_dram_parameter("ident", [128, 128], F32, isOutput=False)
    onesm_ext = nc.declare_dram_parameter("onesm", [128, 128], BF16, isOutput=False)
    out_ext = nc.declare_dram_parameter("out", [N, CS], F16, isOutput=True)

    import contextlib

    ctx = contextlib.ExitStack()
    sb = lambda name, shape, dt=F32: ctx.enter_context(nc.sbuf_tensor(name, shape, dt))
    ps = lambda name: ctx.enter_context(nc.psum_tensor(name, [128, F], F32))
    sem = lambda name: ctx.enter_context(nc.semaphore(name))

    with ctx:
        # --- SBUF ---
        eT_sb = sb("eT_sb", [128, KD * N], BF16)
        erow_sb = sb("erow_sb", [128, NB * D], BF16)
        wt = [sb(f"wt{b}", [128, KD * F], BF16) for b in range(NWT)]
        wsq = [sb(f"wsq{b}", [128, KD * F], BF16) for b in range(NWSQ)]
        sq_scr = sb("sq_scr", [128, D], BF16)
        esq_acc = sb("esq_acc", [128, NB])
        tmp8 = sb("tmp8", [128, NB])
        rsqe_sb = sb("rsqe_sb", [128, NB])
        lnw_tmp = sb("lnw_tmp", [128, F])
        rsqw_bc = [sb(f"rsqw_bc{b}", [128, F]) for b in range(NBC)]
        outq = [sb(f"outq{b}", [128, 4 * F], F16) for b in range(NOUTQ)]
        outs = [sb(f"outs{b}", [128, F], F16) for b in range(NB)]
        ident_sb = sb("ident_sb", [128, 128])
        onesm_sb = sb("onesm_sb", [128, 128], BF16)
        diag_tmp = sb("diag_tmp", [128, 128])
        vdiag = sb("vdiag", [128, 1])
        sqv = sb("sqv", [128, 1])
        lnu = sb("lnu", [128, 1])
        s3v = sb("s3v", [128, 1])
        t1v = sb("t1v", [128, 1])
        fixp = sb("fixp", [128, 1])
        deltap = sb("deltap", [128, 1])
        lnS_b = sb("lnS_b", [128, 1])
        s2_b = sb("s2_b", [128, 1])

        # --- PSUM: 6 main banks + 2 ssq banks = 8 ---
        ps_main = [ps(f"ps_main{b}") for b in range(NPS)]
        ps_ssq = [ps(f"ps_ssq{b}") for b in range(2)]

        # --- semaphores ---
        s_const = sem("s_const")   # ident + onesm
        s_eT = sem("s_eT")
        s_erow = sem("s_erow")
        s_ms = sem("s_ms")
        s_wt = [sem(f"s_wt{b}") for b in range(NWT)]
        s_wsq = sem("s_wsq")       # DVE squares done (count of seq-chunks)
        s_ssqmm = sem("s_ssqmm")   # PE ssq reduction done
        s_lnw = sem("s_lnw")       # ACT Ln consumed ps_ssq
        s_rw = sem("s_rw")         # rsqw_bc ready
        s_en = sem("s_en")         # rsqe ready (1: nb0, 2: nb1-7)
        s_mmtile = sem("s_mmtile")
        s_evtile = sem("s_evtile")
        s_vg = sem("s_vg")
        s_sfix = sem("s_sfix")
        s_vfix = sem("s_vfix")
        s_do = sem("s_do")         # quad out-DMA completions
        s_do24 = sem("s_do24")     # single (chunk 24) out-DMA completions

        with nc.Block() as block:

            @block.gpsimd
            def _(g):
                g.memset(lnS_b[:], float(np.log(S))).then_inc(s_ms, 1)
                g.memset(s2_b[:], float(S * S)).then_inc(s_ms, 1)

                def wt_dma(o):
                    c = SEQ[o]
                    g.dma_start(
                        out=wt[o % NWT][:],
                        in_=w_ext[:, c * KD * F:(c + 1) * KD * F],
                    ).then_inc(s_wt[o % NWT], 16)

                # e row-major first (unblocks ACT e-norm), then first chunk,
                # then eT (unblocks PE mains)
                g.dma_start(out=erow_sb[:], in_=erow_ext[:]).then_inc(s_erow, 16)
                wt_dma(0)
                g.dma_start(out=eT_sb[:], in_=eT_ext[:]).then_inc(s_eT, 16)
                g.dma_start(out=ident_sb[:], in_=ident_ext[:]).then_inc(s_const, 16)
                g.dma_start(out=onesm_sb[:], in_=onesm_ext[:]).then_inc(s_const, 16)
                for o in range(1, 5):
                    wt_dma(o)
                # chunk-24 singles as their evictions land
                for nb in range(NB):
                    g.wait_ge(s_evtile, nb + 1)
                    g.dma_start(
                        out=out_ext[nb * 128:(nb + 1) * 128, 24 * F:25 * F],
                        in_=outs[nb][:],
                    ).then_inc(s_do24, 16)
                for p in range(NPAIR):
                    # two lookahead wt loads; buffer freed by pair p-1's mains
                    for o in (2 * p + 5, 2 * p + 6):
                        if o <= NCHUNK - 1:
                            g.wait_ge(s_mmtile, _mains_done_tiles(o - NWT))
                            wt_dma(o)
                    if p % 2 == 1:
                        q = p // 2
                        for nb in range(NB):
                            g.wait_ge(s_evtile, 8 + 16 * p + 2 * nb + 2)
                            if q == 0 and nb == 0:
                                g.wait_ge(s_vfix, 1)
                            qi = q * NB + nb
                            g.dma_start(
                                out=out_ext[nb * 128:(nb + 1) * 128,
                                            q * 4 * F:(q + 1) * 4 * F],
                                in_=outq[qi % NOUTQ][:],
                            ).then_inc(s_do, 16)
                g.wait_ge(s_do, 16 * (NPAIR // 2) * NB)
                g.wait_ge(s_do24, 16 * NB)

            @block.scalar
            def _(s):
                s.wait_ge(s_ms, 2)
                s.wait_ge(s_erow, 16)

                def esq_block(nb):
                    return s.activation(
                        sq_scr[:], erow_sb[:, nb * D:(nb + 1) * D], AF.Square,
                        accum_out=esq_acc[:, nb:nb + 1],
                    )

                def rsqe_block(lo, hi):
                    s.drain()
                    s.activation(tmp8[:, lo:hi], esq_acc[:, lo:hi], AF.Ln)
                    s.drain()
                    s.activation(rsqe_sb[:, lo:hi], tmp8[:, lo:hi], AF.Exp,
                                 scale=-0.5, bias=lnS_b[:])
                    return s.drain()

                def w_chain(o):
                    s.wait_ge(s_ssqmm, o + 1)
                    if o >= NBC:
                        s.wait_ge(s_evtile, _evict_done_tiles(o - NBC))
                    s.activation(lnw_tmp[:], ps_ssq[o % 2][:], AF.Ln).then_inc(s_lnw, 1)
                    s.drain()
                    s.activation(rsqw_bc[o % NBC][:], lnw_tmp[:], AF.Exp, scale=-0.5)
                    s.drain().then_inc(s_rw, 1)

                # rsqe staged: nb0 | w-chain(0) | nb1-3 | nb4-7, so early
                # evictions unblock while the rest of e-norm runs
                esq_block(0)
                rsqe_block(0, 1).then_inc(s_en, 1)
                w_chain(0)
                for nb in range(1, 4):
                    esq_block(nb)
                rsqe_block(1, 4).then_inc(s_en, 1)
                for nb in range(4, NB):
                    esq_block(nb)
                rsqe_block(4, NB).then_inc(s_en, 1)
                for o in range(1, NCHUNK):
                    w_chain(o)
                    if o == 2:
                        # margin trig on the gathered diagonal [128,1]
                        s.wait_ge(s_vg, 1)
                        s.activation(sqv[:], vdiag[:], AF.Square)
                        s.drain()
                        s.activation(lnu[:], sqv[:], AF.Ln, scale=-1.0, bias=s2_b[:])
                        s.drain()
                        s.activation(s3v[:], lnu[:], AF.Exp, scale=0.5)
                        s.activation(t1v[:], vdiag[:], AF.Copy, scale=COSM)
                        s.drain().then_inc(s_sfix, 1)

            @block.tensor
            def _(t):
                t.wait_ge(s_const, 32)  # onesm (+ident)

                def do_ssq(o):
                    t.wait_ge(s_wsq, o + 1)
                    if o >= 2:
                        t.wait_ge(s_lnw, o - 1)  # ps_ssq[o%2] freed by Ln(o-2)
                    for k in range(KD):
                        mm = t.matmul(
                            ps_ssq[o % 2][:], lhsT=onesm_sb[:],
                            rhs=wsq[o % NWSQ][:, k * F:(k + 1) * F],
                            start=(k == 0), stop=(k == KD - 1),
                            skip_group_check=True,
                        )
                    mm.then_inc(s_ssqmm, 1)

                def main_tile(et, nb, wbuf):
                    if et >= NPS:
                        t.wait_ge(s_evtile, et - NPS + 1)
                    for k in range(KD):
                        mm = t.matmul(
                            ps_main[et % NPS][:],
                            lhsT=eT_sb[:, k * N + nb * 128:k * N + (nb + 1) * 128],
                            rhs=wt[wbuf][:, k * F:(k + 1) * F],
                            start=(k == 0), stop=(k == KD - 1),
                            skip_group_check=True,
                        )
                    mm.then_inc(s_mmtile, 1)

                # ssq(0) before the eT wait: it only needs wsq(0), and it
                # unblocks the ACT w-chain for the first evictions
                do_ssq(0)
                t.wait_ge(s_eT, 16)
                # chunk 24 (seq 0): 8 tiles, with ssq(1..2) interleaved
                for nb in range(NB):
                    main_tile(nb, nb, 0)
                    if nb == 3:
                        do_ssq(1)
                    if nb == 5:
                        do_ssq(2)
                for p in range(NPAIR):
                    b0 = (2 * p + 1) % NWT
                    b1 = (2 * p + 2) % NWT
                    u0 = (2 * p + 1) // NWT + 1
                    u1 = (2 * p + 2) // NWT + 1
                    t.wait_ge(s_wt[b0], 16 * u0)
                    t.wait_ge(s_wt[b1], 16 * u1)
                    for nb in range(NB):
                        if nb == 0 and 2 * p + 3 <= NCHUNK - 1:
                            do_ssq(2 * p + 3)
                        if nb == 4 and 2 * p + 4 <= NCHUNK - 1:
                            do_ssq(2 * p + 4)
                        et0 = 8 + 16 * p + 2 * nb
                        if et0 + 1 >= NPS:
                            t.wait_ge(s_evtile, et0 + 1 - NPS + 1)
                        for k in range(KD):
                            for j, wbuf in ((0, b0), (1, b1)):
                                mm = t.matmul(
                                    ps_main[(et0 + j) % NPS][:],
                                    lhsT=eT_sb[:, k * N + nb * 128:k * N + (nb + 1) * 128],
                                    rhs=wt[wbuf][:, k * F:(k + 1) * F],
                                    start=(k == 0), stop=(k == KD - 1),
                                    skip_group_check=True,
                                )
                                if k == KD - 1:
                                    mm.then_inc(s_mmtile, 1)

            @block.vector
            def _(v):
                def squares(o):
                    v.wait_ge(s_wt[o % NWT], 16 * (o // NWT + 1))
                    if o >= NWSQ:
                        v.wait_ge(s_ssqmm, o - NWSQ + 1)
                    v.tensor_tensor(out=wsq[o % NWSQ][:], in0=wt[o % NWT][:],
                                    in1=wt[o % NWT][:], op=OP.mult)
                    v.engine_nop().then_inc(s_wsq, 1)

                def evict(et, out_ap, nb, o):
                    v.wait_ge(s_mmtile, et + 1)
                    v.scalar_tensor_tensor(
                        out_ap, ps_main[et % NPS][:], rsqe_sb[:, nb:nb + 1],
                        rsqw_bc[o % NBC][:], OP.mult, OP.mult,
                    )
                    v.engine_nop().then_inc(s_evtile, 1)

                squares(0)
                squares(1)
                squares(2)
                # chunk 24 evictions
                v.wait_ge(s_en, 1)
                v.wait_ge(s_rw, 1)
                for nb in range(NB):
                    if nb == 1:
                        v.wait_ge(s_en, 2)
                    if nb == 4:
                        v.wait_ge(s_en, 3)
                    evict(nb, outs[nb][:], nb, 0)
                    if nb == 2:
                        squares(3)
                    if nb == 5:
                        squares(4)
                for p in range(NPAIR):
                    q, h = p // 2, p % 2
                    for nb in range(NB):
                        if nb == 1 and 2 * p + 5 <= NCHUNK - 1:
                            squares(2 * p + 5)
                        if nb == 5 and 2 * p + 6 <= NCHUNK - 1:
                            squares(2 * p + 6)
                        qi = q * NB + nb
                        for j in range(2):
                            et = 8 + 16 * p + 2 * nb + j
                            o = 2 * p + j + 1
                            if nb == 0:
                                v.wait_ge(s_rw, o + 1)
                            if qi >= NOUTQ and h == 0 and j == 0:
                                v.wait_ge(s_do, 16 * (qi - NOUTQ + 1))
                            col = (2 * h + j) * F
                            evict(et, outq[qi % NOUTQ][:, col:col + F], nb, o)
                            if p == 0 and nb == 0 and j == 0:
                                # extract the label diagonal of tile (0,0)
                                v.drain()
                                v.tensor_tensor(out=diag_tmp[:], in0=outq[0][:, 0:128],
                                                in1=ident_sb[:], op=OP.mult)
                                v.drain()
                                v.tensor_reduce(vdiag[:], diag_tmp[:],
                                                mybir.AxisListType.X, OP.add)
                                v.drain().then_inc(s_vg, 1)
                    if p == 0:
                        # margin rewrite after pair-0 evictions
                        v.wait_ge(s_sfix, 1)
                        v.scalar_tensor_tensor(fixp[:], s3v[:], -SINM, t1v[:],
                                               OP.mult, OP.add)
                        v.drain()
                        v.tensor_tensor(out=deltap[:], in0=fixp[:], in1=vdiag[:],
                                        op=OP.subtract)
                        v.drain()
                        v.scalar_tensor_tensor(outq[0][:, 0:128], ident_sb[:],
                                               deltap[:], outq[0][:, 0:128],
                                               OP.mult, OP.add)
                        v.drain().then_inc(s_vfix, 1)

    return nc


_GRAPH = None


def _get_graph():
    global _GRAPH
    if _GRAPH is None:
        _GRAPH = build_graph()
    return _GRAPH


def _host_prepare(embeddings, weight, labels):
    """Row/class permutations putting each core's labels on the (0,0) diagonal,
    packed into partition-contiguous bf16 layouts."""
    labels = np.asarray(labels).astype(np.int64)
    e = np.asarray(embeddings, dtype=np.float32)
    w = np.asarray(weight, dtype=np.float32)

    first_seen = {}
    extras = []  # (core, col, row, cls) for duplicate label classes
    for i in range(N):
        l = int(labels[i])
        m, p = i // 128, i % 128
        if l not in first_seen:
            first_seen[l] = (m, p)
        else:
            extras.append((m, p, i, l))

    labeled = np.zeros(C, dtype=bool)
    labeled[labels] = True
    unlab = np.nonzero(~labeled)[0]

    colmaps = np.full((NCORES, CS), -1, dtype=np.int64)
    for i in range(N):
        colmaps[i // 128, i % 128] = labels[i]
    fill_slots = NCORES * (CS - 128)
    fill = np.full(fill_slots, -1, dtype=np.int64)
    fill[: unlab.size] = unlab
    fill = fill.reshape(NCORES, CS - 128)
    colmaps[:, 128:] = fill

    valid_bulk = colmaps >= 0
    for (m, p, i, l) in extras:
        valid_bulk[m, p] = False

    e_bf = e.astype(ml_dtypes.bfloat16)
    wTfull = w.T  # [512, 100000] view
    ident = np.eye(128, dtype=np.float32)
    onesm = np.ones((128, 128), dtype=ml_dtypes.bfloat16)
    in_maps = []
    row_perms = []
    for m in range(NCORES):
        cm = colmaps[m]
        wsel = np.zeros((D, CS), dtype=np.float32)
        vmask = cm >= 0
        wsel[:, vmask] = wTfull[:, cm[vmask]]
        wsel[0, ~vmask] = 1.0
        w_bf = wsel.astype(ml_dtypes.bfloat16)
        # [D=(ko,p), CS=(c,f)] -> [p, c, ko, f]
        wprep = np.ascontiguousarray(
            w_bf.reshape(KD, 128, NCHUNK, F).transpose(1, 2, 0, 3)
        ).reshape(128, NCHUNK * KD * F)
        rows = np.concatenate([
            np.arange(m * 128, (m + 1) * 128),
            np.delete(np.arange(N), np.s_[m * 128:(m + 1) * 128]),
        ])
        row_perms.append(rows)
        e_perm = e_bf[rows]                      # [N, D]
        eTprep = np.ascontiguousarray(
            e_perm.T.reshape(KD, 128, N).transpose(1, 0, 2)
        ).reshape(128, KD * N)
        erow = np.ascontiguousarray(
            e_perm.reshape(NB, 128, D).transpose(1, 0, 2)
        ).reshape(128, NB * D)
        in_maps.append({
            "eT": eTprep,
            "erow": erow,
            "w": wprep,
            "ident": ident,
            "onesm": onesm,
        })
    return in_maps, row_perms, colmaps, valid_bulk, extras


def _assemble(results, row_perms, colmaps, valid_bulk, extras):
    out = np.empty((N, C), dtype=np.float32)
    slabs = []
    for m in range(NCORES):
        slab = results[m]["out"].astype(np.float32)
        unperm = np.empty_like(slab)
        unperm[row_perms[m]] = slab
        slabs.append(unperm)
        vb = valid_bulk[m]
        out[:, colmaps[m][vb]] = unperm[:, vb]
    for (m, p, i, l) in extras:
        out[i, l] = slabs[m][i, p]
    return out


def kernel(embeddings, weight, labels, _trace=False):
    nc = _get_graph()
    in_maps, row_perms, colmaps, valid_bulk, extras = _host_prepare(
        embeddings, weight, labels
    )
    res = run_bass_kernel_spmd(nc, in_maps, core_ids=list(range(NCORES)), trace=_trace)
    out = _assemble(res.results, row_perms, colmaps, valid_bulk, extras)
    if _trace:
        return out, res
    return out


# revision 10
# speedup vs baseline: 1.2861x; 1.0512x over previous
"""ArcFace logits on 8 Trainium2 NeuronCores (Bass, raw engine streams).

out[n, c] = S * cos(theta_nc + M * [c == labels[n]]),  cos from L2-normalized
embeddings [1024, 512] x weight [100000, 512].

Model-parallel over the class dim (partial-FC): classes are padded/permuted
on the host so every core gets 12800 columns and its 128 label hits land on
the diagonal of output tile (chunk 0, row-block 0).  The compiled graph is
identical on all 8 cores and label-independent.

v3 schedule:
  - bf16 inputs / fp16 output, host-packed for 4KB+ contiguous descriptors
  - raw-e matmul; both norm scales fused into ONE DVE op per PSUM eviction:
        out = (psum * rsqe[row]) * rsqw[col]
  - PSUM arranged as 3 pair-banks of [128,1024]: one eviction instruction
    covers both chunks of a pair (104 evictions total, half the sem waits)
  - w column norms: DVE squares wt and pre-reduces over the 4 k-tiles
    (3 bf16 adds), so the PE ssq reduction is ONE [128x128]x[128x512]
    matmul per chunk with an all-ones lhsT (result pre-broadcast across
    partitions); ACT does exp(-0.5*ln(ssq)) straight out of PSUM
  - ssq/w-chain run two pairs ahead of consumption (NWT=8) so rsqw never
    gates an eviction
  - e row norms via ACT Square+accum_out on a row-major e copy, staged so
    the first evictions unblock early; ACT tables preloaded with a dummy op
  - the odd chunk (24) runs FIRST so the kernel tail is a full pair with
    overlapped eviction/DMA
"""

import math

import numpy as np
import ml_dtypes

import concourse.bass as bass
import concourse.mybir as mybir
from concourse.bass_utils import run_bass_kernel_spmd

AF = mybir.ActivationFunctionType
OP = mybir.AluOpType
F32 = mybir.dt.float32
F16 = mybir.dt.float16
BF16 = mybir.dt.bfloat16

S = 30.0
MARGIN = 0.5
N, D, C = 1024, 512, 100000

NCORES = 8
CS = 12800            # classes per core (padded: 8 * 12800 = 102400)
F = 512               # matmul free dim / class chunk width
NCHUNK = CS // F      # 25
KD = D // 128         # 4 contraction sub-tiles
NB = N // 128         # 8 row blocks
NWT = 8               # wt chunk buffers
NBCP = 4              # rsqw pair-buffer rotation
NOUTQ = 16            # out quad-buffer rotation
NPAIR = (NCHUNK - 1) // 2  # 12 pairs after the leading single chunk
NU = NB + NPAIR * NB  # 104 eviction units

COSM = float(math.cos(MARGIN))
SINM = float(math.sin(MARGIN))

# chunk processing order: odd chunk 24 first, then pairs (0,1),(2,3),...
SEQ = [24] + list(range(24))


def _mains_units_done(o):
    """s_mmu value once all main units of seq-chunk o are complete."""
    if o <= 0:
        return NB if o == 0 else 0
    return NB + NB * ((o - 1) // 2 + 1)


def _units_done_rwp(r):
    """s_evu value once all evictions using rwp index r are complete."""
    if r == 0:
        return NB
    return NB + NB * r


def build_graph():
    nc = bass.Bass(target_bir_lowering=False)

    eT_ext = nc.declare_dram_parameter("eT", [128, KD * N], BF16, isOutput=False)
    erow_ext = nc.declare_dram_parameter("erow", [128, NB * D], BF16, isOutput=False)
    w_ext = nc.declare_dram_parameter("w", [128, NCHUNK * KD * F], BF16, isOutput=False)
    ident_ext = nc.declare_dram_parameter("ident", [128, 128], F32, isOutput=False)
    onesm_ext = nc.declare_dram_parameter("onesm", [128, 128], BF16, isOutput=False)
    out_ext = nc.declare_dram_parameter("out", [N, CS], F16, isOutput=True)

    import contextlib

    ctx = contextlib.ExitStack()
    sb = lambda name, shape, dt=F32: ctx.enter_context(nc.sbuf_tensor(name, shape, dt))
    sem = lambda name: ctx.enter_context(nc.semaphore(name))

    with ctx:
        # --- SBUF ---
        eT_sb = sb("eT_sb", [128, KD * N], BF16)
        erow_sb = sb("erow_sb", [128, NB * D], BF16)
        wt = [sb(f"wt{b}", [128, KD * F], BF16) for b in range(NWT)]
        wsq_scr = sb("wsq_scr", [128, KD * F], BF16)
        wsum2 = sb("wsum2", [128, F], BF16)
        wsum = [sb(f"wsum{b}", [128, F], BF16) for b in range(2)]
        sq_scr = sb("sq_scr", [128, D], BF16)
        esq_acc = sb("esq_acc", [128, NB])
        tmp8 = sb("tmp8", [128, NB])
        rsqe_sb = sb("rsqe_sb", [128, NB])
        lnw_tmp = sb("lnw_tmp", [128, F])
        rsqw_p = [sb(f"rsqw_p{b}", [128, 2 * F]) for b in range(NBCP)]
        outq = [sb(f"outq{b}", [128, 4 * F], F16) for b in range(NOUTQ)]
        outs = [sb(f"outs{b}", [128, F], F16) for b in range(NB)]
        ident_sb = sb("ident_sb", [128, 128])
        onesm_sb = sb("onesm_sb", [128, 128], BF16)
        diag_tmp = sb("diag_tmp", [128, 128])
        vdiag = sb("vdiag", [128, 1])
        sqv = sb("sqv", [128, 1])
        lnu = sb("lnu", [128, 1])
        s3v = sb("s3v", [128, 1])
        t1v = sb("t1v", [128, 1])
        fixp = sb("fixp", [128, 1])
        deltap = sb("deltap", [128, 1])
        lnS_b = sb("lnS_b", [128, 1])
        s2_b = sb("s2_b", [128, 1])

        # --- PSUM: 3 pair-banks [128,1024] + 2 ssq banks [128,512] = 16KB ---
        ps_pair = [
            ctx.enter_context(nc.psum_tensor(f"ps_pair{b}", [128, 2 * F], F32))
            for b in range(3)
        ]
        ps_ssq = [
            ctx.enter_context(nc.psum_tensor(f"ps_ssq{b}", [128, F], F32))
            for b in range(2)
        ]

        # --- semaphores ---
        s_const = sem("s_const")   # ident + onesm
        s_eT = sem("s_eT")
        s_erow = sem("s_erow")
        s_ms = sem("s_ms")
        s_wt = [sem(f"s_wt{b}") for b in range(NWT)]
        s_wsum = sem("s_wsum")     # DVE square+reduce done (seq-chunk count)
        s_ssqmm = sem("s_ssqmm")   # PE ssq matmul done
        s_lnw = sem("s_lnw")       # ACT Ln consumed ps_ssq
        s_rwp = sem("s_rwp")       # rsqw pair-buffer ready (chunk24=1, pair p=p+2)
        s_en = sem("s_en")         # rsqe ready (1: nb0, 2: nb1-3, 3: nb4-7)
        s_mmu = sem("s_mmu")       # PE unit done
        s_evu = sem("s_evu")       # DVE unit evicted
        s_vg = sem("s_vg")
        s_sfix = sem("s_sfix")
        s_vfix = sem("s_vfix")
        s_do = sem("s_do")         # quad out-DMA completions
        s_do24 = sem("s_do24")     # single (chunk 24) out-DMA completions

        with nc.Block() as block:

            @block.gpsimd
            def _(g):
                g.memset(lnS_b[:], float(np.log(S))).then_inc(s_ms, 1)
                g.memset(s2_b[:], float(S * S)).then_inc(s_ms, 1)

                def wt_dma(o):
                    c = SEQ[o]
                    g.dma_start(
                        out=wt[o % NWT][:],
                        in_=w_ext[:, c * KD * F:(c + 1) * KD * F],
                    ).then_inc(s_wt[o % NWT], 16)

                wt_dma(0)
                g.dma_start(out=erow_sb[:], in_=erow_ext[:]).then_inc(s_erow, 16)
                g.dma_start(out=onesm_sb[:], in_=onesm_ext[:]).then_inc(s_const, 16)
                g.dma_start(out=ident_sb[:], in_=ident_ext[:]).then_inc(s_const, 16)
                g.dma_start(out=eT_sb[:], in_=eT_ext[:]).then_inc(s_eT, 16)
                for o in range(1, 7):
                    wt_dma(o)
                # chunk-24 singles as their evictions land
                for t in range(NB):
                    g.wait_ge(s_evu, t + 1)
                    g.dma_start(
                        out=out_ext[t * 128:(t + 1) * 128, 24 * F:25 * F],
                        in_=outs[t][:],
                    ).then_inc(s_do24, 16)
                for p in range(NPAIR):
                    for o in (2 * p + 7, 2 * p + 8):
                        if o <= NCHUNK - 1:
                            oo = o - NWT
                            if oo >= 0:
                                g.wait_ge(s_mmu, _mains_units_done(oo))
                            wt_dma(o)
                    if p % 2 == 1:
                        q = p // 2
                        for nb in range(NB):
                            g.wait_ge(s_evu, NB + NB * p + nb + 1)
                            if q == 0 and nb == 0:
                                g.wait_ge(s_vfix, 1)
                            qi = q * NB + nb
                            g.dma_start(
                                out=out_ext[nb * 128:(nb + 1) * 128,
                                            q * 4 * F:(q + 1) * 4 * F],
                                in_=outq[qi % NOUTQ][:],
                            ).then_inc(s_do, 16)
                g.wait_ge(s_do, 16 * (NPAIR // 2) * NB)
                g.wait_ge(s_do24, 16 * NB)

            @block.scalar
            def _(s):
                # dummy op: pulls the ACT table load off the critical path
                s.activation(sqv[:], vdiag[:], AF.Square)
                s.wait_ge(s_ms, 2)
                s.wait_ge(s_erow, 16)

                def esq_block(nb):
                    s.activation(
                        sq_scr[:], erow_sb[:, nb * D:(nb + 1) * D], AF.Square,
                        accum_out=esq_acc[:, nb:nb + 1],
                    )

                def rsqe_block(lo, hi):
                    s.drain()
                    s.activation(tmp8[:, lo:hi], esq_acc[:, lo:hi], AF.Ln)
                    s.drain()
                    s.activation(rsqe_sb[:, lo:hi], tmp8[:, lo:hi], AF.Exp,
                                 scale=-0.5, bias=lnS_b[:])
                    return s.drain()

                def w_chain(o, r, idx, last):
                    s.wait_ge(s_ssqmm, o + 1)
                    if r >= NBCP and idx == 0:
                        s.wait_ge(s_evu, _units_done_rwp(r - NBCP))
                    s.activation(lnw_tmp[:], ps_ssq[o % 2][:], AF.Ln).then_inc(s_lnw, 1)
                    s.drain()
                    s.activation(rsqw_p[r % NBCP][:, idx * F:(idx + 1) * F],
                                 lnw_tmp[:], AF.Exp, scale=-0.5)
                    d = s.drain()
                    if last:
                        d.then_inc(s_rwp, 1)

                # rsqe staged nb0 | w-chain(24) | nb1-3 | nb4-7
                esq_block(0)
                rsqe_block(0, 1).then_inc(s_en, 1)
                w_chain(0, 0, 0, True)
                for nb in range(1, 4):
                    esq_block(nb)
                rsqe_block(1, 4).then_inc(s_en, 1)
                for nb in range(4, NB):
                    esq_block(nb)
                rsqe_block(4, NB).then_inc(s_en, 1)
                for p in range(NPAIR):
                    w_chain(2 * p + 1, p + 1, 0, False)
                    w_chain(2 * p + 2, p + 1, 1, True)
                    if p == 0:
                        # margin trig on the gathered diagonal [128,1]
                        s.wait_ge(s_vg, 1)
                        s.activation(sqv[:], vdiag[:], AF.Square)
                        s.drain()
                        s.activation(lnu[:], sqv[:], AF.Ln, scale=-1.0, bias=s2_b[:])
                        s.drain()
                        s.activation(s3v[:], lnu[:], AF.Exp, scale=0.5)
                        s.activation(t1v[:], vdiag[:], AF.Copy, scale=COSM)
                        s.drain().then_inc(s_sfix, 1)

            @block.tensor
            def _(t):
                t.wait_ge(s_const, 32)  # onesm (+ident)

                def do_ssq(o):
                    t.wait_ge(s_wsum, o + 1)
                    if o >= 2:
                        t.wait_ge(s_lnw, o - 1)  # ps_ssq[o%2] freed by Ln(o-2)
                    t.matmul(
                        ps_ssq[o % 2][:], lhsT=onesm_sb[:], rhs=wsum[o % 2][:],
                        start=True, stop=True, skip_group_check=True,
                    ).then_inc(s_ssqmm, 1)

                # ssq(0) before the eT wait: unblocks the first w-chain
                do_ssq(0)
                t.wait_ge(s_eT, 16)
                # chunk 24: 8 single tiles in pair-bank halves
                for tt in range(NB):
                    if tt >= 6:
                        t.wait_ge(s_evu, tt - 5)
                    bank, half = (tt >> 1) % 3, tt & 1
                    for k in range(KD):
                        mm = t.matmul(
                            ps_pair[bank][:, half * F:(half + 1) * F],
                            lhsT=eT_sb[:, k * N + tt * 128:k * N + (tt + 1) * 128],
                            rhs=wt[0][:, k * F:(k + 1) * F],
                            start=(k == 0), stop=(k == KD - 1),
                            skip_group_check=True,
                        )
                    mm.then_inc(s_mmu, 1)
                    if tt == 2:
                        do_ssq(1)
                    if tt == 4:
                        do_ssq(2)
                    if tt == 6:
                        do_ssq(3)
                do_ssq(4)
                for p in range(NPAIR):
                    o0, o1 = 2 * p + 1, 2 * p + 2
                    t.wait_ge(s_wt[o0 % NWT], 16 * (o0 // NWT + 1))
                    t.wait_ge(s_wt[o1 % NWT], 16 * (o1 // NWT + 1))
                    for nb in range(NB):
                        u = NB + NB * p + nb
                        # bank u%3 freed by unit u-3; except u=9 whose bank 0
                        # was last used by chunk-24 tiles 6,7 (units 7,8)
                        t.wait_ge(s_evu, u - 2 if u != 9 else 8)
                        for k in range(KD):
                            for j, ob in ((0, o0), (1, o1)):
                                mm = t.matmul(
                                    ps_pair[u % 3][:, j * F:(j + 1) * F],
                                    lhsT=eT_sb[:, k * N + nb * 128:k * N + (nb + 1) * 128],
                                    rhs=wt[ob % NWT][:, k * F:(k + 1) * F],
                                    start=(k == 0), stop=(k == KD - 1),
                                    skip_group_check=True,
                                )
                        mm.then_inc(s_mmu, 1)
                        if nb == 1 and 2 * p + 5 <= NCHUNK - 1:
                            do_ssq(2 * p + 5)
                        if nb == 5 and 2 * p + 6 <= NCHUNK - 1:
                            do_ssq(2 * p + 6)

            @block.vector
            def _(v):
                def prep(o):
                    v.wait_ge(s_wt[o % NWT], 16 * (o // NWT + 1))
                    if o >= 2:
                        v.wait_ge(s_ssqmm, o - 1)  # wsum[o%2] freed by ssq(o-2)
                    b = o % NWT
                    v.tensor_tensor(out=wsq_scr[:], in0=wt[b][:], in1=wt[b][:],
                                    op=OP.mult)
                    v.tensor_tensor(out=wsum[o % 2][:], in0=wsq_scr[:, 0:F],
                                    in1=wsq_scr[:, F:2 * F], op=OP.add)
                    v.tensor_tensor(out=wsum2[:], in0=wsq_scr[:, 2 * F:3 * F],
                                    in1=wsq_scr[:, 3 * F:4 * F], op=OP.add)
                    v.tensor_tensor(out=wsum[o % 2][:], in0=wsum[o % 2][:],
                                    in1=wsum2[:], op=OP.add).then_inc(s_wsum, 1)

                prep(0)
                prep(1)
                prep(2)
                # chunk 24 evictions (units 0..7)
                v.wait_ge(s_en, 1)
                v.wait_ge(s_rwp, 1)
                for tt in range(NB):
                    if tt == 1:
                        v.wait_ge(s_en, 2)
                    if tt == 4:
                        v.wait_ge(s_en, 3)
                    bank, half = (tt >> 1) % 3, tt & 1
                    v.wait_ge(s_mmu, tt + 1)
                    v.scalar_tensor_tensor(
                        outs[tt][:], ps_pair[bank][:, half * F:(half + 1) * F],
                        rsqe_sb[:, tt:tt + 1], rsqw_p[0][:, 0:F],
                        OP.mult, OP.mult,
                    ).then_inc(s_evu, 1)
                    if tt == 2:
                        prep(3)
                    if tt == 5:
                        prep(4)
                for p in range(NPAIR):
                    q, h = p // 2, p % 2
                    r = p + 1
                    for nb in range(NB):
                        if nb == 0:
                            if 2 * p + 5 <= NCHUNK - 1:
                                prep(2 * p + 5)
                            v.wait_ge(s_rwp, r + 1)
                        if nb == 4 and 2 * p + 6 <= NCHUNK - 1:
                            prep(2 * p + 6)
                        u = NB + NB * p + nb
                        qi = q * NB + nb
                        if qi >= NOUTQ and h == 0:
                            v.wait_ge(s_do, 16 * (qi - NOUTQ + 1))
                        v.wait_ge(s_mmu, u + 1)
                        v.scalar_tensor_tensor(
                            outq[qi % NOUTQ][:, h * 2 * F:(h + 1) * 2 * F],
                            ps_pair[u % 3][:], rsqe_sb[:, nb:nb + 1],
                            rsqw_p[r % NBCP][:], OP.mult, OP.mult,
                        ).then_inc(s_evu, 1)
                        if p == 0 and nb == 0:
                            # extract the label diagonal of tile (0,0)
                            v.drain()
                            v.tensor_tensor(out=diag_tmp[:], in0=outq[0][:, 0:128],
                                            in1=ident_sb[:], op=OP.mult)
                            v.drain()
                            v.tensor_reduce(vdiag[:], diag_tmp[:],
                                            mybir.AxisListType.X, OP.add)
                            v.drain().then_inc(s_vg, 1)
                    if p == 0:
                        # margin rewrite after pair-0 evictions
                        v.wait_ge(s_sfix, 1)
                        v.scalar_tensor_tensor(fixp[:], s3v[:], -SINM, t1v[:],
                                               OP.mult, OP.add)
                        v.drain()
                        v.tensor_tensor(out=deltap[:], in0=fixp[:], in1=vdiag[:],
                                        op=OP.subtract)
                        v.drain()
                        v.scalar_tensor_tensor(outq[0][:, 0:128], ident_sb[:],
                                               deltap[:], outq[0][:, 0:128],
                                               OP.mult, OP.add)
                        v.drain().then_inc(s_vfix, 1)

    return nc


_GRAPH = None


def _get_graph():
    global _GRAPH
    if _GRAPH is None:
        _GRAPH = build_graph()
    return _GRAPH


def _host_prepare(embeddings, weight, labels):
    """Row/class permutations putting each core's labels on the (0,0) diagonal,
    packed into partition-contiguous bf16 layouts."""
    labels = np.asarray(labels).astype(np.int64)
    e = np.asarray(embeddings, dtype=np.float32)
    w = np.asarray(weight, dtype=np.float32)

    first_seen = {}
    extras = []  # (core, col, row, cls) for duplicate label classes
    for i in range(N):
        l = int(labels[i])
        m, p = i // 128, i % 128
        if l not in first_seen:
            first_seen[l] = (m, p)
        else:
            extras.append((m, p, i, l))

    labeled = np.zeros(C, dtype=bool)
    labeled[labels] = True
    unlab = np.nonzero(~labeled)[0]

    colmaps = np.full((NCORES, CS), -1, dtype=np.int64)
    for i in range(N):
        colmaps[i // 128, i % 128] = labels[i]
    fill_slots = NCORES * (CS - 128)
    fill = np.full(fill_slots, -1, dtype=np.int64)
    fill[: unlab.size] = unlab
    fill = fill.reshape(NCORES, CS - 128)
    colmaps[:, 128:] = fill

    valid_bulk = colmaps >= 0
    for (m, p, i, l) in extras:
        valid_bulk[m, p] = False

    e_bf = e.astype(ml_dtypes.bfloat16)
    wTfull = w.T  # [512, 100000] view
    ident = np.eye(128, dtype=np.float32)
    onesm = np.ones((128, 128), dtype=ml_dtypes.bfloat16)
    in_maps = []
    row_perms = []
    for m in range(NCORES):
        cm = colmaps[m]
        wsel = np.zeros((D, CS), dtype=np.float32)
        vmask = cm >= 0
        wsel[:, vmask] = wTfull[:, cm[vmask]]
        wsel[0, ~vmask] = 1.0
        w_bf = wsel.astype(ml_dtypes.bfloat16)
        # [D=(ko,p), CS=(c,f)] -> [p, c, ko, f]
        wprep = np.ascontiguousarray(
            w_bf.reshape(KD, 128, NCHUNK, F).transpose(1, 2, 0, 3)
        ).reshape(128, NCHUNK * KD * F)
        rows = np.concatenate([
            np.arange(m * 128, (m + 1) * 128),
            np.delete(np.arange(N), np.s_[m * 128:(m + 1) * 128]),
        ])
        row_perms.append(rows)
        e_perm = e_bf[rows]                      # [N, D]
        eTprep = np.ascontiguousarray(
            e_perm.T.reshape(KD, 128, N).transpose(1, 0, 2)
        ).reshape(128, KD * N)
        erow = np.ascontiguousarray(
            e_perm.reshape(NB, 128, D).transpose(1, 0, 2)
        ).reshape(128, NB * D)
        in_maps.append({
            "eT": eTprep,
            "erow": erow,
            "w": wprep,
            "ident": ident,
            "onesm": onesm,
        })
    return in_maps, row_perms, colmaps, valid_bulk, extras


def _assemble(results, row_perms, colmaps, valid_bulk, extras):
    out = np.empty((N, C), dtype=np.float32)
    slabs = []
    for m in range(NCORES):
        slab = results[m]["out"].astype(np.float32)
        unperm = np.empty_like(slab)
        unperm[row_perms[m]] = slab
        slabs.append(unperm)
        vb = valid_bulk[m]
        out[:, colmaps[m][vb]] = unperm[:, vb]
    for (m, p, i, l) in extras:
        out[i, l] = slabs[m][i, p]
    return out


def kernel(embeddings, weight, labels, _trace=False):
    nc = _get_graph()
    in_maps, row_perms, colmaps, valid_bulk, extras = _host_prepare(
        embeddings, weight, labels
    )
    res = run_bass_kernel_spmd(nc, in_maps, core_ids=list(range(NCORES)), trace=_trace)
    out = _assemble(res.results, row_perms, colmaps, valid_bulk, extras)
    if _trace:
        return out, res
    return out
